# revision 1
# baseline (speedup 1.0000x reference)
"""BidLatte (linear-attention) Trainium2 kernel, 8-core SPMD.

Math (per batch b):
  K = X@Wk; Q = X@Wq; E = exp(K)*mask          (max-shift cancels exactly)
  Ksum = sum_t E;  KX = E^T @ X                (L x D state, avoids X@Wv)
  KXn = KX / Ksum; Kv = KXn @ Wv; Kv_bd = blockdiag_head(Kv)
  G = Kv_bd @ o_proj                           (o_proj folded into state)
  out = softmax_head(Q) @ G

Sharding: core 2i+j -> batch i, T-half j. One pairwise AllReduce of the
(L x D+1) state per batch pair.

X is fed twice in bf16 (natural + host-pre-transposed) so no on-chip
transposition of X is needed. All matmuls run in bf16 (full PE rate;
fp32/fp32r matmuls lower to 2-pass half-rate on TRN2). Elementwise /
exp / reductions and all PSUM accumulation stay fp32. Per-element bf16
errors average out in the global T-reduction; measured output rel err
vs the fp32 reference is ~5e-3. Dummy matmuls around the AllReduce keep
the PE HAM clock at 2.4GHz through the phase transition.
"""
import numpy as np

_B, _T, _D, _L, _H = 4, 8192, 1024, 128, 16
NCORES = 8
TLOC = _T // 2  # tokens per core
BT = 512        # tokens per block
NBLK = TLOC // BT
NT = BT // 128  # t-tiles per block
DC = _D // 128  # d-chunks

_cache = {}


def _build():
    import concourse.bacc as bacc
    import concourse.mybir as mybir
    import concourse.tile as tile

    FP32 = mybir.dt.float32
    FP32R = mybir.dt.float32r
    BF16 = mybir.dt.bfloat16
    EXP = mybir.ActivationFunctionType.Exp

    nc = bacc.Bacc("TRN2", target_bir_lowering=False, debug=False,
                   num_devices=NCORES)

    xs = nc.dram_tensor("xs", [TLOC, _D], BF16, kind="ExternalInput")
    xst = nc.dram_tensor("xst", [_D, TLOC], BF16, kind="ExternalInput")
    ms = nc.dram_tensor("ms", [128, TLOC // 128], FP32, kind="ExternalInput")
    wk = nc.dram_tensor("wk", [128, _D], BF16, kind="ExternalInput")
    wq = nc.dram_tensor("wq", [128, _D], BF16, kind="ExternalInput")
    wv = nc.dram_tensor("wv", [128, DC * _D], BF16, kind="ExternalInput")
    op = nc.dram_tensor("op", [128, DC * _D], BF16, kind="ExternalInput")
    ident = nc.dram_tensor("ident", [128, 128], BF16, kind="ExternalInput")
    ph = nc.dram_tensor("ph", [128, _H], BF16, kind="ExternalInput")
    pht = nc.dram_tensor("pht", [_H, 128], BF16, kind="ExternalInput")
    ones2 = nc.dram_tensor("ones2", [128, 2], BF16, kind="ExternalInput")
    bdm = nc.dram_tensor("bdm", [128, _D], FP32, kind="ExternalInput")
    out = nc.dram_tensor("out", [TLOC, _D], FP32, kind="ExternalOutput")

    with tile.TileContext(nc) as tc:
        with (
            tc.tile_pool(name="const", bufs=1) as cpool,
            tc.tile_pool(name="dram", bufs=1, space="DRAM") as dpool,
        ):
            wk_sb = cpool.tile([128, _D], BF16)
            wq_sb = cpool.tile([128, _D], BF16)
            wv_sb = cpool.tile([128, DC * _D], BF16)
            op_sb = cpool.tile([128, DC * _D], BF16)
            id_sb = cpool.tile([128, 128], BF16)
            ph_sb = cpool.tile([128, _H], BF16)
            pht_sb = cpool.tile([_H, 128], BF16)
            on_sb = cpool.tile([128, 2], BF16)
            ms_sb = cpool.tile([128, TLOC // 128], FP32)
            bdm_sb = cpool.tile([128, _D], FP32)
            qst_sb = cpool.tile([128, TLOC], BF16)   # persistent softmax(Q)^T
            kxp_sb = cpool.tile([128, 1032], FP32)    # packed KX | Ksum
            kxr_sb = cpool.tile([128, 1032], FP32)    # reduced state
            g_sb = cpool.tile([128, _D], BF16)       # folded output weights


            ar_in = dpool.tile([128, 1032], FP32)
            ar_out = dpool.tile([128, 1032], FP32)

            # ---------------- Phase A: state + softmax(Q)^T ----------------
            with (
                tc.tile_pool(name="xin", bufs=4) as xin,
                tc.tile_pool(name="xtin", bufs=4) as xtin,
                tc.tile_pool(name="esb", bufs=3) as esb,
                tc.tile_pool(name="e2", bufs=8) as e2p,
                tc.tile_pool(name="srp", bufs=2) as srp,
                tc.tile_pool(name="scr_ps", bufs=3, space="PSUM") as scr,
                tc.tile_pool(name="kt_ps", bufs=1, space="PSUM") as ktp,
                tc.tile_pool(name="qt_ps", bufs=1, space="PSUM") as qtp,
                tc.tile_pool(name="kx_ps", bufs=1, space="PSUM") as kxp,
                tc.tile_pool(name="ks_ps", bufs=1, space="PSUM") as ksp,
            ):
                kx_ps = kxp.tile([128, _D], FP32)
                ks_ps = ksp.tile([128, 2], FP32)

                def stage2(k, xts, et, eq):
                    """softmax + E-transpose + KX/KS accumulation, block k."""
                    s_ps = scr.tile([_H, BT], FP32, tag="scr")
                    nc.tensor.matmul(s_ps[:], ph_sb[:], eq[:], start=True,
                                     stop=True)
                    sr = srp.tile([_H, BT], FP32, tag="sr")
                    nc.vector.reciprocal_approx_fast(sr[:], s_ps[:])
                    srb = srp.tile([_H, BT], BF16, tag="srb")
                    nc.vector.tensor_copy(srb[:], sr[:])
                    bq_ps = scr.tile([128, BT], FP32, tag="scr")
                    nc.tensor.matmul(bq_ps[:], pht_sb[:], srb[:], start=True,
                                     stop=True)
                    nc.vector.tensor_mul(
                        qst_sb[:, k * BT:(k + 1) * BT], eq[:], bq_ps[:]
                    )
                    e_ps = scr.tile([128, BT], BF16, tag="scr")
                    for i in range(NT):
                        nc.tensor.transpose(
                            e_ps[:, i * 128:(i + 1) * 128],
                            et[:, i * 128:(i + 1) * 128],
                            id_sb[:],
                        )
                    for i in range(NT):
                        e2 = e2p.tile([128, 128], BF16, tag="e2")
                        j = k * NT + i
                        nc.vector.tensor_scalar_mul(
                            e2[:], e_ps[:, i * 128:(i + 1) * 128],
                            ms_sb[:, j:j + 1],
                        )
                        first = (k == 0 and i == 0)
                        last = (k == NBLK - 1 and i == NT - 1)
                        nc.tensor.matmul(kx_ps[:, 0:512], e2[:],
                                         xts[i][:, 0:512],
                                         start=first, stop=last)
                        nc.tensor.matmul(kx_ps[:, 512:1024], e2[:],
                                         xts[i][:, 512:1024],
                                         start=first, stop=last)
                        nc.tensor.matmul(ks_ps[:], e2[:], on_sb[:],
                                         start=first, stop=last)

                prev = None
                for k in range(NBLK):
                    if 1 <= k <= 4:
                        for cc2 in range(2):
                            c2 = (k - 1) * 2 + cc2
                            nc.scalar.dma_start(
                                out=wv_sb[:, c2 * _D:(c2 + 1) * _D],
                                in_=wv.ap()[:, c2 * _D:(c2 + 1) * _D])
                    if 3 <= k <= 6:
                        for cc2 in range(2):
                            c2 = (k - 3) * 2 + cc2
                            nc.scalar.dma_start(
                                out=op_sb[:, c2 * _D:(c2 + 1) * _D],
                                in_=op.ap()[:, c2 * _D:(c2 + 1) * _D])
                    xblk = xin.tile([128, NT * _D], BF16, tag="xin")
                    nc.sync.dma_start(
                        out=xblk[:].rearrange("p (a d) -> p a d", a=NT),
                        in_=xs.ap()[k * BT:(k + 1) * BT, :].rearrange(
                            "(a p) d -> p a d", p=128))
                    xts = [xblk[:, i * _D:(i + 1) * _D] for i in range(NT)]
                    xtblk = xtin.tile([128, DC * BT], BF16, tag="xtin")
                    nc.sync.dma_start(
                        out=xtblk[:].rearrange("p (c t) -> p c t", c=DC),
                        in_=xst.ap().rearrange("(c p) t -> p c t", p=128)
                        [:, :, k * BT:(k + 1) * BT])
                    xtts = [xtblk[:, c * BT:(c + 1) * BT] for c in range(DC)]

                    if k == 0:
                        nc.sync.dma_start(out=wk_sb[:], in_=wk.ap())
                        nc.sync.dma_start(out=wq_sb[:], in_=wq.ap())
                        nc.scalar.dma_start(out=id_sb[:], in_=ident.ap())
                        nc.scalar.dma_start(out=ph_sb[:], in_=ph.ap())
                        nc.scalar.dma_start(out=pht_sb[:], in_=pht.ap())
                        nc.scalar.dma_start(out=on_sb[:], in_=ones2.ap())
                        nc.scalar.dma_start(out=ms_sb[:], in_=ms.ap())
                    if k == 1:
                        nc.scalar.dma_start(out=bdm_sb[:], in_=bdm.ap())

                    if k == 0:
                        for w in range(14):
                            wt0 = scr.tile([128, BT], FP32, tag="scr")
                            nc.tensor.matmul(
                                wt0[:], wk_sb[:, 0:128],
                                wk_sb[:, 0:512], start=True, stop=True)

                    kt_ps = ktp.tile([128, BT], FP32)
                    qt_ps = qtp.tile([128, BT], FP32)
                    for c in range(DC):
                        nc.tensor.matmul(
                            kt_ps[:], wk_sb[:, c * 128:(c + 1) * 128],
                            xtts[c][:],
                            start=(c == 0), stop=(c == DC - 1),
                        )
                        nc.tensor.matmul(
                            qt_ps[:], wq_sb[:, c * 128:(c + 1) * 128],
                            xtts[c][:],
                            start=(c == 0), stop=(c == DC - 1),
                        )

                    et = esb.tile([128, BT], BF16, tag="et")
                    nc.scalar.activation(et[:], kt_ps[:], EXP)
                    eq = esb.tile([128, BT], BF16, tag="eq")
                    nc.scalar.activation(eq[:], qt_ps[:], EXP)

                    if prev is not None:
                        stage2(*prev)
                    prev = (k, xts, et, eq)
                stage2(*prev)

                # pack state for the collective
                nc.vector.tensor_copy(kxp_sb[:, 0:512], kx_ps[:, 0:512])
                nc.scalar.copy(kxp_sb[:, 512:1024], kx_ps[:, 512:1024])
                nc.vector.tensor_copy(kxp_sb[:, 1024:1025], ks_ps[:, 0:1])
                nc.vector.memset(kxp_sb[:, 1025:1032], 0.0)

            nc.sync.dma_start(out=ar_in[:], in_=kxp_sb[:])
            nc.gpsimd.collective_compute(
                "AllReduce",
                mybir.AluOpType.add,
                replica_groups=[[0, 1], [2, 3], [4, 5], [6, 7]],
                ins=[ar_in.opt()],
                outs=[ar_out.opt()],
            )
            nc.sync.dma_start(out=kxr_sb[:], in_=ar_out[:])

            # keep the PE clock warm while the AllReduce is in flight,
            # then a dependent burst that re-warms it right as the reduced
            # state arrives (so phase B/C run at full clock)
            with tc.tile_pool(name="warm_ps", bufs=1, space="PSUM") as wps:
                wtile = wps.tile([128, BT], FP32)
                for w in range(45):
                    nc.tensor.matmul(
                        wtile[:], wk_sb[:, 0:128],
                        qst_sb[:, 0:BT],
                        start=True, stop=True)
                for w in range(12):
                    nc.tensor.matmul(
                        wtile[:],
                        kxr_sb[:, 0:64].bitcast(BF16)[:, 0:128],
                        qst_sb[:, 0:BT],
                        start=True, stop=True)

            # ---------------- Phase B: G = blockdiag(KXn @ Wv) @ o_proj ----
            with (
                tc.tile_pool(name="bsb", bufs=2) as bsb,
                tc.tile_pool(name="bsb1", bufs=1) as bsb1,
                tc.tile_pool(name="bps_small", bufs=2, space="PSUM") as bpss,
                tc.tile_pool(name="bps_big", bufs=2, space="PSUM") as bpsb,
            ):
                rk = bsb1.tile([128, 1], FP32)
                nc.vector.reciprocal_approx_fast(rk[:], kxr_sb[:, 1024:1025])
                kxn = bsb1.tile([128, _D], BF16)
                nc.vector.tensor_scalar_mul(kxn[:], kxr_sb[:, 0:1024], rk[:])

                kxnt = []
                for c in range(DC):
                    tp = bpss.tile([128, 128], BF16, tag="btp")
                    nc.tensor.transpose(tp[:], kxn[:, c * 128:(c + 1) * 128],
                                        id_sb[:])
                    t_sb = bsb.tile([128, 128], BF16, tag="bts")
                    nc.vector.tensor_copy(t_sb[:], tp[:])
                    kxnt.append(t_sb)

                kv_ps = bpsb.tile([128, _D], FP32, tag="big")
                for c in range(DC):
                    nc.tensor.matmul(
                        kv_ps[:, 0:512], kxnt[c][:],
                        wv_sb[:, c * _D:c * _D + 512],
                        start=(c == 0), stop=(c == DC - 1))
                    nc.tensor.matmul(
                        kv_ps[:, 512:1024], kxnt[c][:],
                        wv_sb[:, c * _D + 512:(c + 1) * _D],
                        start=(c == 0), stop=(c == DC - 1))

                # block-diagonal extract via 0/1 mask multiply
                kvbd = bsb1.tile([128, _D], BF16)
                nc.vector.tensor_mul(kvbd[:], kv_ps[:], bdm_sb[:])
                kvbdt = []
                for c in range(DC):
                    tp = bpss.tile([128, 128], BF16, tag="btpf")
                    nc.tensor.transpose(tp[:], kvbd[:, c * 128:(c + 1) * 128],
                                        id_sb[:])
                    t_sb = bsb.tile([128, 128], BF16, tag="btsf")
                    nc.vector.tensor_copy(t_sb[:], tp[:])
                    kvbdt.append(t_sb)

                g_ps = bpsb.tile([128, _D], FP32, tag="big")
                for c in range(DC):
                    nc.tensor.matmul(
                        g_ps[:, 0:512], kvbdt[c][:],
                        op_sb[:, c * _D:c * _D + 512],
                        start=(c == 0), stop=(c == DC - 1))
                    nc.tensor.matmul(
                        g_ps[:, 512:1024], kvbdt[c][:],
                        op_sb[:, c * _D + 512:(c + 1) * _D],
                        start=(c == 0), stop=(c == DC - 1))
                nc.vector.tensor_copy(g_sb[:, 0:512], g_ps[:, 0:512])
                nc.scalar.copy(g_sb[:, 512:1024], g_ps[:, 512:1024])

            # ---------------- Phase C: out = Qs @ G ------------------------
            with (
                tc.tile_pool(name="osb", bufs=3) as osb,
                tc.tile_pool(name="ops", bufs=3, space="PSUM") as ops,
                tc.tile_pool(name="wps2", bufs=1, space="PSUM") as wps2,
            ):
                wtile2 = wps2.tile([128, BT], FP32)
                for w in range(12):
                    nc.tensor.matmul(wtile2[:], g_sb[:, 0:128],
                                     qst_sb[:, 0:BT], start=True, stop=True)
                for j in range(TLOC // 256):
                    ot = osb.tile([128, 2 * _D], FP32, tag="osb")
                    for h2 in range(2):
                        i = 2 * j + h2
                        o_ps = ops.tile([128, _D], FP32, tag="ops")
                        lhs = qst_sb[:, i * 128:(i + 1) * 128]
                        nc.tensor.matmul(o_ps[:, 0:512], lhs, g_sb[:, 0:512],
                                         start=True, stop=True)
                        nc.tensor.matmul(o_ps[:, 512:1024], lhs,
                                         g_sb[:, 512:1024], start=True,
                                         stop=True)
                        dst = ot[:, h2 * _D:(h2 + 1) * _D]
                        if i % 2 == 0:
                            nc.vector.tensor_copy(dst, o_ps[:])
                        else:
                            nc.scalar.copy(dst, o_ps[:])
                    nc.sync.dma_start(
                        out=out.ap()[j * 256:(j + 1) * 256, :].rearrange(
                            "(a p) d -> p a d", p=128),
                        in_=ot[:].rearrange("p (a d) -> p a d", a=2))

    nc.compile()
    return nc


def _host_inputs(X, attention_mask, Wk, Wq, Wv, o_proj):
    import ml_dtypes

    BF = ml_dtypes.bfloat16
    X = np.asarray(X, dtype=np.float32)
    mask = np.asarray(attention_mask, dtype=np.float32)
    Wk = np.asarray(Wk, dtype=np.float32)
    Wq = np.asarray(Wq, dtype=np.float32)
    Wv = np.asarray(Wv, dtype=np.float32)
    o_proj = np.asarray(o_proj, dtype=np.float32)

    wk_r = np.ascontiguousarray(
        Wk.reshape(DC, 128, _L).transpose(1, 0, 2).reshape(128, DC * _L)
    ).astype(BF)
    wq_r = np.ascontiguousarray(
        Wq.reshape(DC, 128, _L).transpose(1, 0, 2).reshape(128, DC * _L)
    ).astype(BF)
    wv_r = np.ascontiguousarray(
        Wv.reshape(DC, 128, _D).transpose(1, 0, 2).reshape(128, DC * _D)
    ).astype(BF)
    op_r = np.ascontiguousarray(
        o_proj.reshape(DC, 128, _D).transpose(1, 0, 2).reshape(128, DC * _D)
    ).astype(BF)
    ident = np.eye(128, dtype=BF)
    ph_m = np.zeros((128, _H), dtype=BF)
    for hh in range(_H):
        ph_m[hh * (_L // _H):(hh + 1) * (_L // _H), hh] = 1.0
    pht_m = np.ascontiguousarray(ph_m.T)
    ones2 = np.ones((128, 2), dtype=BF)
    bdm_m = np.zeros((128, _D), dtype=np.float32)
    for hh in range(_H):
        bdm_m[hh * (_L // _H):(hh + 1) * (_L // _H),
              hh * (_D // _H):(hh + 1) * (_D // _H)] = 1.0

    Xbf = X.astype(BF)
    in_maps = []
    for core in range(NCORES):
        b, half = core // 2, core % 2
        xsh = np.ascontiguousarray(Xbf[b, half * TLOC:(half + 1) * TLOC, :])
        xsth = np.ascontiguousarray(xsh.T)
        msh = np.ascontiguousarray(
            mask[b, half * TLOC:(half + 1) * TLOC]
            .reshape(TLOC // 128, 128).T)
        in_maps.append({
            "xs": xsh, "xst": xsth, "ms": msh, "wk": wk_r, "wq": wq_r,
            "wv": wv_r, "op": op_r, "ident": ident, "ph": ph_m,
            "pht": pht_m, "ones2": ones2, "bdm": bdm_m,
        })
    return in_maps


def _run(in_maps, trace=False):
    from concourse.bass_utils import run_bass_kernel_spmd

    if "nc" not in _cache:
        _cache["nc"] = _build()
    return run_bass_kernel_spmd(
        _cache["nc"], in_maps, list(range(NCORES)), trace=trace)


def kernel(X, attention_mask, Wk, Wq, Wv, o_proj, n_heads=16):
    in_maps = _host_inputs(X, attention_mask, Wk, Wq, Wv, o_proj)
    res = _run(in_maps)
    out = np.empty((_B, _T, _D), dtype=np.float32)
    for core in range(NCORES):
        b, half = core // 2, core % 2
        out[b, half * TLOC:(half + 1) * TLOC, :] = res.results[core]["out"]
    return out



# revision 3
# speedup vs baseline: 1.2296x; 1.2296x over previous
"""BidLatte (linear-attention) Trainium2 kernel, 8-core SPMD.

Math (per batch b):
  K = X@Wk; Q = X@Wq; E = exp(K)*mask          (max-shift cancels exactly)
  Ksum = sum_t E;  KX = E^T @ X                (L x D state, avoids X@Wv)
  Kv = KX @ Wv; Kv_bd = blockdiag_head(Kv)
  G = (Kv_bd @ o_proj) / Ksum[:,None]          (1/Ksum folded into G evac)
  out = softmax_head(Q) @ G

Sharding: core 2i+j -> batch i, T-half j. One pairwise AllReduce of the
bf16-packed (128 x 1026) state (KX | Ksum) per batch pair.

X is fed twice (natural + host-pre-transposed), both pre-tiled so each
512-token block is one contiguous 1MB DMA. All matmuls run in bf16;
elementwise / exp / reductions and PSUM accumulation stay fp32. Output
is written bf16 and cast to fp32 on the host. Dummy matmuls issued
AFTER the collective trigger keep the PE HAM clock warm through the
AllReduce without delaying its entry barrier.
"""
import numpy as np

_B, _T, _D, _L, _H = 4, 8192, 1024, 128, 16
NCORES = 8
TLOC = _T // 2  # tokens per core
BT = 512        # tokens per block
NBLK = TLOC // BT
NT = BT // 128  # t-tiles per block
DC = _D // 128  # d-chunks
AR_DUMMIES = 56  # PE warm-keepers issued after the collective trigger

_cache = {}


def _build():
    import concourse.bacc as bacc
    import concourse.mybir as mybir
    import concourse.tile as tile

    FP32 = mybir.dt.float32
    BF16 = mybir.dt.bfloat16
    EXP = mybir.ActivationFunctionType.Exp
    COPY = mybir.ActivationFunctionType.Copy

    nc = bacc.Bacc("TRN2", target_bir_lowering=False, debug=False,
                   num_devices=NCORES)

    xs = nc.dram_tensor("xs", [128, NBLK * NT * _D], BF16,
                        kind="ExternalInput")
    xst = nc.dram_tensor("xst", [128, NBLK * DC * BT], BF16,
                         kind="ExternalInput")
    ms = nc.dram_tensor("ms", [128, TLOC // 128], FP32, kind="ExternalInput")
    wk = nc.dram_tensor("wk", [128, _D], BF16, kind="ExternalInput")
    wq = nc.dram_tensor("wq", [128, _D], BF16, kind="ExternalInput")
    wv = nc.dram_tensor("wv", [128, DC * _D], BF16, kind="ExternalInput")
    op = nc.dram_tensor("op", [128, DC * _D], BF16, kind="ExternalInput")
    ident = nc.dram_tensor("ident", [128, 128], BF16, kind="ExternalInput")
    ph = nc.dram_tensor("ph", [128, _H], BF16, kind="ExternalInput")
    pht = nc.dram_tensor("pht", [_H, 128], BF16, kind="ExternalInput")
    ones2 = nc.dram_tensor("ones2", [128, 2], BF16, kind="ExternalInput")
    bdm = nc.dram_tensor("bdm", [128, _D], BF16, kind="ExternalInput")
    out = nc.dram_tensor("out", [128, (TLOC // 128) * _D], BF16,
                         kind="ExternalOutput")

    with tile.TileContext(nc) as tc:
        with (
            tc.tile_pool(name="const", bufs=1) as cpool,
            tc.tile_pool(name="dram", bufs=1, space="DRAM") as dpool,
        ):
            wk_sb = cpool.tile([128, _D], BF16)
            wq_sb = cpool.tile([128, _D], BF16)
            wv_sb = cpool.tile([128, DC * _D], BF16)
            op_sb = cpool.tile([128, DC * _D], BF16)
            id_sb = cpool.tile([128, 128], BF16)
            ph_sb = cpool.tile([128, _H], BF16)
            pht_sb = cpool.tile([_H, 128], BF16)
            on_sb = cpool.tile([128, 2], BF16)
            ms_sb = cpool.tile([128, TLOC // 128], FP32)
            bdm_sb = cpool.tile([128, _D], BF16)
            qst_sb = cpool.tile([128, TLOC], BF16)   # persistent softmax(Q)^T
            kxp_sb = cpool.tile([128, 1026], BF16)   # packed KX | Ksum
            kxr_sb = cpool.tile([128, 1026], BF16)   # reduced state
            g_sb = cpool.tile([128, _D], BF16)       # folded output weights
            wrm_sb = cpool.tile([128, BT], BF16)     # PE warm-up scratch

            ar_in = dpool.tile([128, 1026], BF16)
            ar_out = dpool.tile([128, 1026], BF16)

            # ---------------- Phase A: state + softmax(Q)^T ----------------
            with (
                tc.tile_pool(name="xin", bufs=4) as xin,
                tc.tile_pool(name="xtin", bufs=4) as xtin,
                tc.tile_pool(name="esb", bufs=3) as esb,
                tc.tile_pool(name="e2", bufs=8) as e2p,
                tc.tile_pool(name="srp", bufs=2) as srp,
                tc.tile_pool(name="scr_ps", bufs=3, space="PSUM") as scr,
                tc.tile_pool(name="kt_ps", bufs=1, space="PSUM") as ktp,
                tc.tile_pool(name="qt_ps", bufs=1, space="PSUM") as qtp,
                tc.tile_pool(name="kx_ps", bufs=1, space="PSUM") as kxp,
                tc.tile_pool(name="ks_ps", bufs=1, space="PSUM") as ksp,
            ):
                kx_ps = kxp.tile([128, _D], FP32)
                ks_ps = ksp.tile([128, 2], FP32)

                # weights first on the scalar ring so the first projection
                # matmuls are gated only by the (parallel) sync-ring x^T DMA
                nc.scalar.dma_start(out=wk_sb[:], in_=wk.ap())
                nc.scalar.dma_start(out=wq_sb[:], in_=wq.ap())
                nc.scalar.dma_start(out=ms_sb[:], in_=ms.ap())
                nc.scalar.dma_start(out=id_sb[:], in_=ident.ap())
                nc.scalar.dma_start(out=ph_sb[:], in_=ph.ap())
                nc.scalar.dma_start(out=pht_sb[:], in_=pht.ap())
                nc.scalar.dma_start(out=on_sb[:], in_=ones2.ap())

                # PE warm-up with no DMA dependency: memset scratch + matmuls
                nc.vector.memset(wrm_sb[:], 0.0)
                for w in range(14):
                    wt0 = scr.tile([128, BT], FP32, tag="scr")
                    nc.tensor.matmul(wt0[:], wrm_sb[:, 0:128], wrm_sb[:],
                                     start=True, stop=True)

                def stage2(k, xts, et, eq):
                    """softmax + E-transpose + KX/KS accumulation, block k."""
                    s_ps = scr.tile([_H, BT], FP32, tag="scr")
                    nc.tensor.matmul(s_ps[:], ph_sb[:], eq[:], start=True,
                                     stop=True)
                    sr = srp.tile([_H, BT], FP32, tag="sr")
                    nc.vector.reciprocal_approx_fast(sr[:], s_ps[:])
                    srb = srp.tile([_H, BT], BF16, tag="srb")
                    nc.vector.tensor_copy(srb[:], sr[:])
                    bq_ps = scr.tile([128, BT], FP32, tag="scr")
                    nc.tensor.matmul(bq_ps[:], pht_sb[:], srb[:], start=True,
                                     stop=True)
                    nc.vector.tensor_mul(
                        qst_sb[:, k * BT:(k + 1) * BT], eq[:], bq_ps[:]
                    )
                    e_ps = scr.tile([128, BT], BF16, tag="scr")
                    for i in range(NT):
                        nc.tensor.transpose(
                            e_ps[:, i * 128:(i + 1) * 128],
                            et[:, i * 128:(i + 1) * 128],
                            id_sb[:],
                        )
                    for i in range(NT):
                        e2 = e2p.tile([128, 128], BF16, tag="e2")
                        j = k * NT + i
                        nc.vector.tensor_scalar_mul(
                            e2[:], e_ps[:, i * 128:(i + 1) * 128],
                            ms_sb[:, j:j + 1],
                        )
                        first = (k == 0 and i == 0)
                        last = (k == NBLK - 1 and i == NT - 1)
                        nc.tensor.matmul(kx_ps[:, 0:512], e2[:],
                                         xts[:, i * _D:i * _D + 512],
                                         start=first, stop=last)
                        nc.tensor.matmul(kx_ps[:, 512:1024], e2[:],
                                         xts[:, i * _D + 512:(i + 1) * _D],
                                         start=first, stop=last)
                        nc.tensor.matmul(ks_ps[:], e2[:], on_sb[:],
                                         start=first, stop=last)

                prev = None
                for k in range(NBLK):
                    if 1 <= k <= 4:
                        for cc2 in range(2):
                            c2 = (k - 1) * 2 + cc2
                            nc.scalar.dma_start(
                                out=wv_sb[:, c2 * _D:(c2 + 1) * _D],
                                in_=wv.ap()[:, c2 * _D:(c2 + 1) * _D])
                    if 3 <= k <= 6:
                        for cc2 in range(2):
                            c2 = (k - 3) * 2 + cc2
                            nc.scalar.dma_start(
                                out=op_sb[:, c2 * _D:(c2 + 1) * _D],
                                in_=op.ap()[:, c2 * _D:(c2 + 1) * _D])
                    if k == 1:
                        nc.scalar.dma_start(out=bdm_sb[:], in_=bdm.ap())

                    # x^T first (gates the projections), then x-natural
                    xtblk = xtin.tile([128, DC * BT], BF16, tag="xtin")
                    nc.sync.dma_start(
                        out=xtblk[:],
                        in_=xst.ap()[:, k * DC * BT:(k + 1) * DC * BT])
                    xblk = xin.tile([128, NT * _D], BF16, tag="xin")
                    nc.sync.dma_start(
                        out=xblk[:],
                        in_=xs.ap()[:, k * NT * _D:(k + 1) * NT * _D])

                    kt_ps = ktp.tile([128, BT], FP32)
                    qt_ps = qtp.tile([128, BT], FP32)
                    for c in range(DC):
                        nc.tensor.matmul(
                            kt_ps[:], wk_sb[:, c * 128:(c + 1) * 128],
                            xtblk[:, c * BT:(c + 1) * BT],
                            start=(c == 0), stop=(c == DC - 1),
                        )
                        nc.tensor.matmul(
                            qt_ps[:], wq_sb[:, c * 128:(c + 1) * 128],
                            xtblk[:, c * BT:(c + 1) * BT],
                            start=(c == 0), stop=(c == DC - 1),
                        )

                    et = esb.tile([128, BT], BF16, tag="et")
                    nc.scalar.activation(et[:], kt_ps[:], EXP)
                    eq = esb.tile([128, BT], BF16, tag="eq")
                    nc.scalar.activation(eq[:], qt_ps[:], EXP)

                    if prev is not None:
                        stage2(*prev)
                    prev = (k, xblk, et, eq)
                stage2(*prev)

                # pack state (bf16) for the collective
                nc.vector.tensor_copy(kxp_sb[:, 0:512], kx_ps[:, 0:512])
                nc.scalar.copy(kxp_sb[:, 512:1024], kx_ps[:, 512:1024])
                nc.vector.tensor_copy(kxp_sb[:, 1024:1025], ks_ps[:, 0:1])
                nc.vector.memset(kxp_sb[:, 1025:1026], 0.0)

            nc.sync.dma_start(out=ar_in[:], in_=kxp_sb[:])
            nc.gpsimd.collective_compute(
                "AllReduce",
                mybir.AluOpType.add,
                replica_groups=[[0, 1], [2, 3], [4, 5], [6, 7]],
                ins=[ar_in.opt()],
                outs=[ar_out.opt()],
            )
            nc.sync.dma_start(out=kxr_sb[:], in_=ar_out[:])

            # keep the PE clock warm while the AllReduce is in flight;
            # issued AFTER the collective so its entry barrier is not
            # stuck behind them in the tensor queue
            with tc.tile_pool(name="warm_ps", bufs=1, space="PSUM") as wps:
                wtile = wps.tile([128, BT], FP32)
                for w in range(AR_DUMMIES):
                    nc.tensor.matmul(
                        wtile[:], wk_sb[:, 0:128],
                        qst_sb[:, 0:BT],
                        start=True, stop=True)

            # ---------------- Phase B: G = blockdiag(KX @ Wv) @ o_proj -----
            with (
                tc.tile_pool(name="bsb", bufs=2) as bsb,
                tc.tile_pool(name="bsb1", bufs=1) as bsb1,
                tc.tile_pool(name="bps_small", bufs=2, space="PSUM") as bpss,
                tc.tile_pool(name="bps_big", bufs=2, space="PSUM") as bpsb,
            ):
                ksf = bsb1.tile([128, 1], FP32)
                nc.vector.tensor_copy(ksf[:], kxr_sb[:, 1024:1025])
                rk = bsb1.tile([128, 1], FP32)
                nc.vector.reciprocal_approx_fast(rk[:], ksf[:])

                kxnt = []
                for c in range(DC):
                    tp = bpss.tile([128, 128], BF16, tag="btp")
                    nc.tensor.transpose(tp[:],
                                        kxr_sb[:, c * 128:(c + 1) * 128],
                                        id_sb[:])
                    t_sb = bsb.tile([128, 128], BF16, tag="bts")
                    if c % 2 == 0:
                        nc.vector.tensor_copy(t_sb[:], tp[:])
                    else:
                        nc.scalar.copy(t_sb[:], tp[:])
                    kxnt.append(t_sb)

                kv_ps = bpsb.tile([128, _D], FP32, tag="big")
                for c in range(DC):
                    nc.tensor.matmul(
                        kv_ps[:, 0:512], kxnt[c][:],
                        wv_sb[:, c * _D:c * _D + 512],
                        start=(c == 0), stop=(c == DC - 1))
                    nc.tensor.matmul(
                        kv_ps[:, 512:1024], kxnt[c][:],
                        wv_sb[:, c * _D + 512:(c + 1) * _D],
                        start=(c == 0), stop=(c == DC - 1))

                # block-diagonal extract via 0/1 mask multiply
                kvbd = bsb1.tile([128, _D], BF16)
                nc.vector.tensor_mul(kvbd[:], kv_ps[:], bdm_sb[:])
                kvbdt = []
                for c in range(DC):
                    tp = bpss.tile([128, 128], BF16, tag="btpf")
                    nc.tensor.transpose(tp[:], kvbd[:, c * 128:(c + 1) * 128],
                                        id_sb[:])
                    t_sb = bsb.tile([128, 128], BF16, tag="btsf")
                    if c % 2 == 0:
                        nc.vector.tensor_copy(t_sb[:], tp[:])
                    else:
                        nc.scalar.copy(t_sb[:], tp[:])
                    kvbdt.append(t_sb)

                g_ps = bpsb.tile([128, _D], FP32, tag="big")
                for c in range(DC):
                    nc.tensor.matmul(
                        g_ps[:, 0:512], kvbdt[c][:],
                        op_sb[:, c * _D:c * _D + 512],
                        start=(c == 0), stop=(c == DC - 1))
                    nc.tensor.matmul(
                        g_ps[:, 512:1024], kvbdt[c][:],
                        op_sb[:, c * _D + 512:(c + 1) * _D],
                        start=(c == 0), stop=(c == DC - 1))
                # 1/Ksum folded into the evacuation (per-partition scale)
                nc.vector.tensor_scalar_mul(g_sb[:, 0:512], g_ps[:, 0:512],
                                            rk[:])
                nc.scalar.activation(g_sb[:, 512:1024], g_ps[:, 512:1024],
                                     COPY, scale=rk[:])

            # ---------------- Phase C: out = Qs @ G ------------------------
            with (
                tc.tile_pool(name="osb", bufs=3) as osb,
                tc.tile_pool(name="ops", bufs=3, space="PSUM") as ops,
            ):
                for j in range(TLOC // BT):
                    ot = osb.tile([128, NT * _D], BF16, tag="osb")
                    for i2 in range(NT):
                        i = NT * j + i2
                        o_ps = ops.tile([128, _D], FP32, tag="ops")
                        lhs = qst_sb[:, i * 128:(i + 1) * 128]
                        nc.tensor.matmul(o_ps[:, 0:512], lhs, g_sb[:, 0:512],
                                         start=True, stop=True)
                        nc.tensor.matmul(o_ps[:, 512:1024], lhs,
                                         g_sb[:, 512:1024], start=True,
                                         stop=True)
                        dst = ot[:, i2 * _D:(i2 + 1) * _D]
                        if i2 % 2 == 0:
                            nc.vector.tensor_copy(dst, o_ps[:])
                        else:
                            nc.scalar.copy(dst, o_ps[:])
                    nc.sync.dma_start(
                        out=out.ap()[:, j * NT * _D:(j + 1) * NT * _D],
                        in_=ot[:])

    nc.compile()
    return nc


def _host_inputs(X, attention_mask, Wk, Wq, Wv, o_proj):
    import ml_dtypes

    BF = ml_dtypes.bfloat16
    X = np.asarray(X, dtype=np.float32)
    mask = np.asarray(attention_mask, dtype=np.float32)
    Wk = np.asarray(Wk, dtype=np.float32)
    Wq = np.asarray(Wq, dtype=np.float32)
    Wv = np.asarray(Wv, dtype=np.float32)
    o_proj = np.asarray(o_proj, dtype=np.float32)

    wk_r = np.ascontiguousarray(
        Wk.reshape(DC, 128, _L).transpose(1, 0, 2).reshape(128, DC * _L)
    ).astype(BF)
    wq_r = np.ascontiguousarray(
        Wq.reshape(DC, 128, _L).transpose(1, 0, 2).reshape(128, DC * _L)
    ).astype(BF)
    wv_r = np.ascontiguousarray(
        Wv.reshape(DC, 128, _D).transpose(1, 0, 2).reshape(128, DC * _D)
    ).astype(BF)
    op_r = np.ascontiguousarray(
        o_proj.reshape(DC, 128, _D).transpose(1, 0, 2).reshape(128, DC * _D)
    ).astype(BF)
    ident = np.eye(128, dtype=BF)
    ph_m = np.zeros((128, _H), dtype=BF)
    for hh in range(_H):
        ph_m[hh * (_L // _H):(hh + 1) * (_L // _H), hh] = 1.0
    pht_m = np.ascontiguousarray(ph_m.T)
    ones2 = np.ones((128, 2), dtype=BF)
    bdm_m = np.zeros((128, _D), dtype=BF)
    for hh in range(_H):
        bdm_m[hh * (_L // _H):(hh + 1) * (_L // _H),
              hh * (_D // _H):(hh + 1) * (_D // _H)] = 1.0

    Xbf = X.astype(BF)
    in_maps = []
    for core in range(NCORES):
        b, half = core // 2, core % 2
        xh = Xbf[b, half * TLOC:(half + 1) * TLOC, :]      # [TLOC, D]
        # pre-tiled natural: (p, k, a, d) = X[k*BT + a*128 + p, d]
        xs_t = np.ascontiguousarray(
            xh.reshape(NBLK, NT, 128, _D).transpose(2, 0, 1, 3)
            .reshape(128, NBLK * NT * _D))
        # pre-tiled transposed: (p, k, c, t) = X[k*BT + t, c*128 + p]
        xst_t = np.ascontiguousarray(
            xh.reshape(NBLK, BT, DC, 128).transpose(3, 0, 2, 1)
            .reshape(128, NBLK * DC * BT))
        msh = np.ascontiguousarray(
            mask[b, half * TLOC:(half + 1) * TLOC]
            .reshape(TLOC // 128, 128).T)
        in_maps.append({
            "xs": xs_t, "xst": xst_t, "ms": msh, "wk": wk_r, "wq": wq_r,
            "wv": wv_r, "op": op_r, "ident": ident, "ph": ph_m,
            "pht": pht_m, "ones2": ones2, "bdm": bdm_m,
        })
    return in_maps


def _run(in_maps, trace=False):
    from concourse.bass_utils import run_bass_kernel_spmd

    if "nc" not in _cache:
        _cache["nc"] = _build()
    return run_bass_kernel_spmd(
        _cache["nc"], in_maps, list(range(NCORES)), trace=trace)


def kernel(X, attention_mask, Wk, Wq, Wv, o_proj, n_heads=16):
    in_maps = _host_inputs(X, attention_mask, Wk, Wq, Wv, o_proj)
    res = _run(in_maps)
    out = np.empty((_B, _T, _D), dtype=np.float32)
    for core in range(NCORES):
        b, half = core // 2, core % 2
        o = np.asarray(res.results[core]["out"]).astype(np.float32)
        out[b, half * TLOC:(half + 1) * TLOC, :] = (
            o.reshape(128, TLOC // 128, _D).transpose(1, 0, 2)
            .reshape(TLOC, _D))
    return out


# revision 11
# speedup vs baseline: 1.5101x; 1.2281x over previous
"""BidLatte (linear-attention) Trainium2 kernel, 8-core SPMD.

Math (per batch b):
  K = X@Wk; Q = X@Wq; E = exp(K)*mask          (max-shift cancels exactly)
  Ksum = sum_t E;  KX = E^T @ X                (L x D state, avoids X@Wv)
  Kv = KX @ Wv; Kv_bd = blockdiag_head(Kv)
  G = (Kv_bd @ o_proj) / Ksum[:,None]          (1/Ksum folded into G evac)
  out = softmax_head(Q) @ G

Sharding: core 2i+j -> batch i, T-half j. One pairwise AllReduce of the
bf16-packed (128 x 1026) state (KX | Ksum) per batch pair.

X is fed twice (natural + host-pre-transposed), both pre-tiled so each
512-token block is one contiguous 1MB DMA. All matmuls run in bf16;
elementwise / exp / reductions and PSUM accumulation stay fp32. Output
is written bf16 and cast to fp32 on the host. Dummy matmuls issued
AFTER the collective trigger keep the PE HAM clock warm through the
AllReduce without delaying its entry barrier.
"""
import numpy as np

_B, _T, _D, _L, _H = 4, 8192, 1024, 128, 16
NCORES = 8
TLOC = _T // 2  # tokens per core
BT = 512        # tokens per block
NBLK = TLOC // BT
NT = BT // 128  # t-tiles per block
DC = _D // 128  # d-chunks
AR_DUMMIES = 12  # PE warm-keepers issued after the exchange trigger
USE_RDMA = True  # pairwise SWDGE remote-DMA state exchange (vs AllReduce)
PK = 1028       # packed state width (4 x 257-col remote-DMA slices)

_cache = {}


def _build():
    import concourse.bacc as bacc
    import concourse.mybir as mybir
    import concourse.tile as tile

    FP32 = mybir.dt.float32
    BF16 = mybir.dt.bfloat16
    EXP = mybir.ActivationFunctionType.Exp
    COPY = mybir.ActivationFunctionType.Copy

    nc = bacc.Bacc("TRN2", target_bir_lowering=False, debug=False,
                   num_devices=NCORES)
    rdma_fixups = []  # (instruction, sem, value): HW waits added post-schedule

    xs = nc.dram_tensor("xs", [128, NBLK * NT * _D], BF16,
                        kind="ExternalInput")
    xst = nc.dram_tensor("xst", [128, NBLK * DC * BT], BF16,
                         kind="ExternalInput")
    ms = nc.dram_tensor("ms", [128, TLOC // 128], FP32, kind="ExternalInput")
    wk = nc.dram_tensor("wk", [128, _D], BF16, kind="ExternalInput")
    wq = nc.dram_tensor("wq", [128, _D], BF16, kind="ExternalInput")
    wv = nc.dram_tensor("wv", [128, DC * _D], BF16, kind="ExternalInput")
    op = nc.dram_tensor("op", [128, DC * _D], BF16, kind="ExternalInput")
    ident = nc.dram_tensor("ident", [128, 128], BF16, kind="ExternalInput")
    ph = nc.dram_tensor("ph", [128, _H], BF16, kind="ExternalInput")
    pht = nc.dram_tensor("pht", [_H, 128], BF16, kind="ExternalInput")
    ones2 = nc.dram_tensor("ones2", [128, 2], BF16, kind="ExternalInput")
    bdm = nc.dram_tensor("bdm", [128, _D], BF16, kind="ExternalInput")
    out = nc.dram_tensor("out", [128, (TLOC // 128) * _D], BF16,
                         kind="ExternalOutput")

    with tile.TileContext(nc) as tc:
        with (
            tc.tile_pool(name="const", bufs=1) as cpool,
            tc.tile_pool(name="dram", bufs=1, space="DRAM") as dpool,
        ):
            wk_sb = cpool.tile([128, _D], BF16)
            wq_sb = cpool.tile([128, _D], BF16)
            wv_sb = cpool.tile([128, DC * _D], BF16)
            op_sb = cpool.tile([128, DC * _D], BF16)
            id_sb = cpool.tile([128, 128], BF16)
            ph_sb = cpool.tile([128, _H], BF16)
            pht_sb = cpool.tile([_H, 128], BF16)
            on_sb = cpool.tile([128, 2], BF16)
            ms_sb = cpool.tile([128, TLOC // 128], FP32)
            bdm_sb = cpool.tile([128, _D], BF16)
            qst_sb = cpool.tile([128, TLOC], BF16)   # persistent softmax(Q)^T
            kxp_sb = cpool.tile([128, PK], BF16)     # packed KX | Ksum
            peer_sb = cpool.tile([128, PK], BF16)    # peer's state (remote)
            kxr_sb = cpool.tile([128, PK], BF16)     # reduced state
            g_sb = cpool.tile([128, _D], BF16)       # folded output weights
            wrm_sb = cpool.tile([128, BT], BF16)     # PE warm-up scratch

            if not USE_RDMA:
                ar_in = dpool.tile([128, PK], BF16)
                ar_out = dpool.tile([128, PK], BF16)

            # ---------------- Phase A: state + softmax(Q)^T ----------------
            with (
                tc.tile_pool(name="xin", bufs=4) as xin,
                tc.tile_pool(name="xtin", bufs=4) as xtin,
                tc.tile_pool(name="esb", bufs=3) as esb,
                tc.tile_pool(name="e2", bufs=8) as e2p,
                tc.tile_pool(name="srp", bufs=2) as srp,
                tc.tile_pool(name="scr_ps", bufs=3, space="PSUM") as scr,
                tc.tile_pool(name="kt_ps", bufs=1, space="PSUM") as ktp,
                tc.tile_pool(name="qt_ps", bufs=1, space="PSUM") as qtp,
                tc.tile_pool(name="kx_ps", bufs=1, space="PSUM") as kxp,
                tc.tile_pool(name="ks_ps", bufs=1, space="PSUM") as ksp,
            ):
                kx_ps = kxp.tile([128, _D], FP32)
                ks_ps = ksp.tile([128, 2], FP32)

                # wk/wq on the scalar ring (gate the first projections, then
                # the ACT queue is reserved for activations); everything else
                # on the sync ring ahead of the first x^T block
                nc.scalar.dma_start(out=wk_sb[:], in_=wk.ap())
                nc.scalar.dma_start(out=wq_sb[:], in_=wq.ap())
                nc.sync.dma_start(out=ms_sb[:], in_=ms.ap())
                nc.sync.dma_start(out=id_sb[:], in_=ident.ap())
                nc.sync.dma_start(out=ph_sb[:], in_=ph.ap())
                nc.sync.dma_start(out=pht_sb[:], in_=pht.ap())
                nc.sync.dma_start(out=on_sb[:], in_=ones2.ap())

                # PE warm-up with no DMA dependency: memset scratch + matmuls
                nc.vector.memset(wrm_sb[:], 0.0)
                for w in range(14):
                    wt0 = scr.tile([128, BT], FP32, tag="scr")
                    nc.tensor.matmul(wt0[:], wrm_sb[:, 0:128], wrm_sb[:],
                                     start=True, stop=True)

                def stage2(k, xts, et, eq):
                    """softmax + E-transpose + KX/KS accumulation, block k."""
                    s_ps = scr.tile([_H, BT], FP32, tag="scr")
                    nc.tensor.matmul(s_ps[:], ph_sb[:], eq[:], start=True,
                                     stop=True)
                    sr = srp.tile([_H, BT], FP32, tag="sr")
                    nc.vector.reciprocal_approx_fast(sr[:], s_ps[:])
                    srb = srp.tile([_H, BT], BF16, tag="srb")
                    nc.vector.tensor_copy(srb[:], sr[:])
                    bq_ps = scr.tile([128, BT], FP32, tag="scr")
                    nc.tensor.matmul(bq_ps[:], pht_sb[:], srb[:], start=True,
                                     stop=True)
                    nc.vector.tensor_mul(
                        qst_sb[:, k * BT:(k + 1) * BT], eq[:], bq_ps[:]
                    )
                    e_ps = scr.tile([128, BT], BF16, tag="scr")
                    for i in range(NT):
                        nc.tensor.transpose(
                            e_ps[:, i * 128:(i + 1) * 128],
                            et[:, i * 128:(i + 1) * 128],
                            id_sb[:],
                        )
                    for i in range(NT):
                        e2 = e2p.tile([128, 128], BF16, tag="e2")
                        j = k * NT + i
                        nc.vector.tensor_scalar_mul(
                            e2[:], e_ps[:, i * 128:(i + 1) * 128],
                            ms_sb[:, j:j + 1],
                        )
                        first = (k == 0 and i == 0)
                        last = (k == NBLK - 1 and i == NT - 1)
                        nc.tensor.matmul(kx_ps[:, 0:512], e2[:],
                                         xts[:, i * _D:i * _D + 512],
                                         start=first, stop=last)
                        nc.tensor.matmul(kx_ps[:, 512:1024], e2[:],
                                         xts[:, i * _D + 512:(i + 1) * _D],
                                         start=first, stop=last)
                        nc.tensor.matmul(ks_ps[:], e2[:], on_sb[:],
                                         start=first, stop=last)

                prev = None
                for k in range(NBLK):
                    # x^T first (gates the projections), then x-natural
                    xtblk = xtin.tile([128, DC * BT], BF16, tag="xtin")
                    nc.sync.dma_start(
                        out=xtblk[:],
                        in_=xst.ap()[:, k * DC * BT:(k + 1) * DC * BT])
                    xblk = xin.tile([128, NT * _D], BF16, tag="xin")
                    nc.sync.dma_start(
                        out=xblk[:],
                        in_=xs.ap()[:, k * NT * _D:(k + 1) * NT * _D])

                    # weight prefetch rides the sync ring behind the x blocks
                    if 1 <= k <= 4:
                        for cc2 in range(2):
                            c2 = (k - 1) * 2 + cc2
                            nc.sync.dma_start(
                                out=wv_sb[:, c2 * _D:(c2 + 1) * _D],
                                in_=wv.ap()[:, c2 * _D:(c2 + 1) * _D])
                    if 3 <= k <= 6:
                        for cc2 in range(2):
                            c2 = (k - 3) * 2 + cc2
                            nc.sync.dma_start(
                                out=op_sb[:, c2 * _D:(c2 + 1) * _D],
                                in_=op.ap()[:, c2 * _D:(c2 + 1) * _D])
                    if k == 1:
                        nc.sync.dma_start(out=bdm_sb[:], in_=bdm.ap())

                    kt_ps = ktp.tile([128, BT], FP32)
                    qt_ps = qtp.tile([128, BT], FP32)
                    for c in range(DC):
                        nc.tensor.matmul(
                            kt_ps[:], wk_sb[:, c * 128:(c + 1) * 128],
                            xtblk[:, c * BT:(c + 1) * BT],
                            start=(c == 0), stop=(c == DC - 1),
                        )
                        nc.tensor.matmul(
                            qt_ps[:], wq_sb[:, c * 128:(c + 1) * 128],
                            xtblk[:, c * BT:(c + 1) * BT],
                            start=(c == 0), stop=(c == DC - 1),
                        )

                    et = esb.tile([128, BT], BF16, tag="et")
                    nc.scalar.activation(et[:], kt_ps[:], EXP)
                    eq = esb.tile([128, BT], BF16, tag="eq")
                    nc.scalar.activation(eq[:], qt_ps[:], EXP)

                    if prev is not None:
                        stage2(*prev)
                    prev = (k, xblk, et, eq)
                stage2(*prev)

                # pack state (bf16) for the exchange
                nc.vector.tensor_copy(kxp_sb[:, 0:512], kx_ps[:, 0:512])
                nc.scalar.copy(kxp_sb[:, 512:1024], kx_ps[:, 512:1024])
                nc.vector.tensor_copy(kxp_sb[:, 1024:1025], ks_ps[:, 0:1])
                nc.vector.memset(kxp_sb[:, 1025:PK], 0.0)

            if USE_RDMA:
                # pairwise SBUF->SBUF exchange with the partner core
                # (relative dest Δtpb=1 -> tpb XOR 1). 4 slices in distinct
                # slots -> 4 disjoint engine pairs carry them concurrently.
                # Each non-None dest bumps the peer's rsem by 16/8 = 2.
                rsem = nc.alloc_semaphore("rdma_state")
                lsem = nc.alloc_semaphore("rdma_local")
                NSL = PK // 257
                for s in range(NSL):
                    rdests = [None] * 8
                    rdests[s] = (0, 1)
                    nc.gpsimd.remote_dma_broadcast(
                        out_ap=peer_sb[:, s * 257:(s + 1) * 257],
                        in_ap=kxp_sb[:, s * 257:(s + 1) * 257],
                        remote_sem=rsem, local_sem=lsem,
                        rdests=rdests)
                nc.gpsimd.trigger_dma(count=None)
            else:
                nc.sync.dma_start(out=ar_in[:], in_=kxp_sb[:])
                nc.gpsimd.collective_compute(
                    "AllReduce",
                    mybir.AluOpType.add,
                    replica_groups=[[0, 1], [2, 3], [4, 5], [6, 7]],
                    ins=[ar_in.opt()],
                    outs=[ar_out.opt()],
                )
                nc.sync.dma_start(out=kxr_sb[:], in_=ar_out[:])

            # keep the PE clock warm while the exchange is in flight;
            # issued AFTER the trigger so it is not stuck behind them
            with tc.tile_pool(name="warm_ps", bufs=1, space="PSUM") as wps:
                wtile = wps.tile([128, BT], FP32)
                for w in range(AR_DUMMIES):
                    nc.tensor.matmul(
                        wtile[:], wk_sb[:, 0:128],
                        qst_sb[:, 0:BT],
                        start=True, stop=True)

            if USE_RDMA:
                # the wait on the peer's arrival sem is attached AFTER Tile's
                # scheduling pass (its single-core sim cannot see the remote
                # increment and would report a deadlock); on HW the add then
                # blocks until the peer's 4 slices have landed
                add_ins = nc.vector.tensor_add(kxr_sb[:], kxp_sb[:],
                                               peer_sb[:])
                rdma_fixups.append((add_ins, rsem, 2 * NSL))

            # ---------------- Phase B: G = blockdiag(KX @ Wv) @ o_proj -----
            with (
                tc.tile_pool(name="bsb", bufs=2) as bsb,
                tc.tile_pool(name="bsb1", bufs=1) as bsb1,
                tc.tile_pool(name="bps_small", bufs=2, space="PSUM") as bpss,
                tc.tile_pool(name="bps_big", bufs=2, space="PSUM") as bpsb,
            ):
                ksf = bsb1.tile([128, 1], FP32)
                nc.vector.tensor_copy(ksf[:], kxr_sb[:, 1024:1025])
                rk = bsb1.tile([128, 1], FP32)
                nc.vector.reciprocal_approx_fast(rk[:], ksf[:])

                kxnt = []
                for c in range(DC):
                    tp = bpss.tile([128, 128], BF16, tag="btp")
                    nc.tensor.transpose(tp[:],
                                        kxr_sb[:, c * 128:(c + 1) * 128],
                                        id_sb[:])
                    t_sb = bsb.tile([128, 128], BF16, tag="bts")
                    if c % 2 == 0:
                        nc.vector.tensor_copy(t_sb[:], tp[:])
                    else:
                        nc.scalar.copy(t_sb[:], tp[:])
                    kxnt.append(t_sb)

                kv_ps = bpsb.tile([128, _D], FP32, tag="big")
                for c in range(DC):
                    nc.tensor.matmul(
                        kv_ps[:, 0:512], kxnt[c][:],
                        wv_sb[:, c * _D:c * _D + 512],
                        start=(c == 0), stop=(c == DC - 1))
                    nc.tensor.matmul(
                        kv_ps[:, 512:1024], kxnt[c][:],
                        wv_sb[:, c * _D + 512:(c + 1) * _D],
                        start=(c == 0), stop=(c == DC - 1))

                # block-diagonal extract via 0/1 mask multiply
                kvbd = bsb1.tile([128, _D], BF16)
                nc.vector.tensor_mul(kvbd[:], kv_ps[:], bdm_sb[:])
                kvbdt = []
                for c in range(DC):
                    tp = bpss.tile([128, 128], BF16, tag="btpf")
                    nc.tensor.transpose(tp[:], kvbd[:, c * 128:(c + 1) * 128],
                                        id_sb[:])
                    t_sb = bsb.tile([128, 128], BF16, tag="btsf")
                    if c % 2 == 0:
                        nc.vector.tensor_copy(t_sb[:], tp[:])
                    else:
                        nc.scalar.copy(t_sb[:], tp[:])
                    kvbdt.append(t_sb)

                g_ps = bpsb.tile([128, _D], FP32, tag="big")
                for c in range(DC):
                    nc.tensor.matmul(
                        g_ps[:, 0:512], kvbdt[c][:],
                        op_sb[:, c * _D:c * _D + 512],
                        start=(c == 0), stop=(c == DC - 1))
                    nc.tensor.matmul(
                        g_ps[:, 512:1024], kvbdt[c][:],
                        op_sb[:, c * _D + 512:(c + 1) * _D],
                        start=(c == 0), stop=(c == DC - 1))
                # 1/Ksum folded into the evacuation (per-partition scale)
                nc.vector.tensor_scalar_mul(g_sb[:, 0:512], g_ps[:, 0:512],
                                            rk[:])
                nc.scalar.activation(g_sb[:, 512:1024], g_ps[:, 512:1024],
                                     COPY, scale=rk[:])

            # ---------------- Phase C: out = Qs @ G ------------------------
            with (
                tc.tile_pool(name="osb", bufs=3) as osb,
                tc.tile_pool(name="ops", bufs=3, space="PSUM") as ops,
            ):
                for j in range(TLOC // BT):
                    ot = osb.tile([128, NT * _D], BF16, tag="osb")
                    for i2 in range(NT):
                        i = NT * j + i2
                        o_ps = ops.tile([128, _D], FP32, tag="ops")
                        lhs = qst_sb[:, i * 128:(i + 1) * 128]
                        nc.tensor.matmul(o_ps[:, 0:512], lhs, g_sb[:, 0:512],
                                         start=True, stop=True)
                        nc.tensor.matmul(o_ps[:, 512:1024], lhs,
                                         g_sb[:, 512:1024], start=True,
                                         stop=True)
                        dst = ot[:, i2 * _D:(i2 + 1) * _D]
                        if i2 % 2 == 0:
                            nc.vector.tensor_copy(dst, o_ps[:])
                        else:
                            nc.scalar.copy(dst, o_ps[:])
                    nc.sync.dma_start(
                        out=out.ap()[:, j * NT * _D:(j + 1) * NT * _D],
                        in_=ot[:])

    for ins, sem, val in rdma_fixups:
        ins.wait_op(sem, val, "sem-ge", check=False)
    nc.compile()
    return nc


def _host_inputs(X, attention_mask, Wk, Wq, Wv, o_proj):
    import ml_dtypes

    BF = ml_dtypes.bfloat16
    X = np.asarray(X, dtype=np.float32)
    mask = np.asarray(attention_mask, dtype=np.float32)
    Wk = np.asarray(Wk, dtype=np.float32)
    Wq = np.asarray(Wq, dtype=np.float32)
    Wv = np.asarray(Wv, dtype=np.float32)
    o_proj = np.asarray(o_proj, dtype=np.float32)

    wk_r = np.ascontiguousarray(
        Wk.reshape(DC, 128, _L).transpose(1, 0, 2).reshape(128, DC * _L)
    ).astype(BF)
    wq_r = np.ascontiguousarray(
        Wq.reshape(DC, 128, _L).transpose(1, 0, 2).reshape(128, DC * _L)
    ).astype(BF)
    wv_r = np.ascontiguousarray(
        Wv.reshape(DC, 128, _D).transpose(1, 0, 2).reshape(128, DC * _D)
    ).astype(BF)
    op_r = np.ascontiguousarray(
        o_proj.reshape(DC, 128, _D).transpose(1, 0, 2).reshape(128, DC * _D)
    ).astype(BF)
    ident = np.eye(128, dtype=BF)
    ph_m = np.zeros((128, _H), dtype=BF)
    for hh in range(_H):
        ph_m[hh * (_L // _H):(hh + 1) * (_L // _H), hh] = 1.0
    pht_m = np.ascontiguousarray(ph_m.T)
    ones2 = np.ones((128, 2), dtype=BF)
    bdm_m = np.zeros((128, _D), dtype=BF)
    for hh in range(_H):
        bdm_m[hh * (_L // _H):(hh + 1) * (_L // _H),
              hh * (_D // _H):(hh + 1) * (_D // _H)] = 1.0

    Xbf = X.astype(BF)
    in_maps = []
    for core in range(NCORES):
        b, half = core // 2, core % 2
        xh = Xbf[b, half * TLOC:(half + 1) * TLOC, :]      # [TLOC, D]
        # pre-tiled natural: (p, k, a, d) = X[k*BT + a*128 + p, d]
        xs_t = np.ascontiguousarray(
            xh.reshape(NBLK, NT, 128, _D).transpose(2, 0, 1, 3)
            .reshape(128, NBLK * NT * _D))
        # pre-tiled transposed: (p, k, c, t) = X[k*BT + t, c*128 + p]
        xst_t = np.ascontiguousarray(
            xh.reshape(NBLK, BT, DC, 128).transpose(3, 0, 2, 1)
            .reshape(128, NBLK * DC * BT))
        msh = np.ascontiguousarray(
            mask[b, half * TLOC:(half + 1) * TLOC]
            .reshape(TLOC // 128, 128).T)
        in_maps.append({
            "xs": xs_t, "xst": xst_t, "ms": msh, "wk": wk_r, "wq": wq_r,
            "wv": wv_r, "op": op_r, "ident": ident, "ph": ph_m,
            "pht": pht_m, "ones2": ones2, "bdm": bdm_m,
        })
    return in_maps


def _run(in_maps, trace=False):
    from concourse.bass_utils import run_bass_kernel_spmd

    if "nc" not in _cache:
        _cache["nc"] = _build()
    return run_bass_kernel_spmd(
        _cache["nc"], in_maps, list(range(NCORES)), trace=trace)


def kernel(X, attention_mask, Wk, Wq, Wv, o_proj, n_heads=16):
    in_maps = _host_inputs(X, attention_mask, Wk, Wq, Wv, o_proj)
    res = _run(in_maps)
    out = np.empty((_B, _T, _D), dtype=np.float32)
    for core in range(NCORES):
        b, half = core // 2, core % 2
        o = np.asarray(res.results[core]["out"]).astype(np.float32)
        out[b, half * TLOC:(half + 1) * TLOC, :] = (
            o.reshape(128, TLOC // 128, _D).transpose(1, 0, 2)
            .reshape(TLOC, _D))
    return out
